# revision 10
# baseline (speedup 1.0000x reference)
"""Trainium2 Bass kernel for nn_MoE_90297392431448.

MoE layer: B=2, T=2048, D=1024, H=4096, E=8 experts, top-K=2 routing.

Strategy (expert-pair parallel, tokens always the streaming free dim):
  - Host: gating softmax + top-2 selection in fp64, renormalized gate
    weights; gather each expert's tokens.
  - Experts are paired big-with-small: pair i -> cores (2i, 2i+1), each
    core takes half of each paired expert's tokens. All cores run ONE
    uniform program (single NEFF, true SPMD) with slot capacities
        capS = ceil(max_small n_e / 2)  (slot 0, first columns)
        capL = ceil(max_e n_e / 2)      (slot 1)
    so per-core work M = capS + capL is within a few %% of the perfect
    balance N_TOK*K/8 -- provably minimal for a uniform program with at
    most two weight sets per core.
  - Device, two-stage FFN with TOKENS as the moving/free dimension in
    BOTH stages (so cost is proportional to M, with no 128-token
    quantization):
        stage 1: ht[h, m] = gelu(sum_k W1[k, h] * xT[k, m] + b1[h])
        stage 2: y[d, m]  = sum_h W2[h, d] * ht[h, m]
    bf16 matmuls, fp32 PSUM accumulation. W1/W2 are streamed through
    SBUF ring buffers in exact consumption order (each weight byte is
    DMA'd once and never kept resident), which is what lets one core
    serve two experts.
  - Host: scale columns by gate weights, scatter-add into [B,T,D], plus
    the w*b2 rank-1 term.
"""

import numpy as np
import ml_dtypes

B, T, D, H, E, K = 2, 2048, 1024, 4096, 8, 2
P = 128
KD = D // P    # 8  k-chunks of the D contraction (stage 1)
CH = H // P    # 32 h-chunks (stage-1 tiles / stage-2 contraction)
DT = D // P    # 8  d-tiles of the output (stage 2)
N_TOK = B * T
N_WARM = 20

_compiled_cache = {}


def _split_cap(cap):
    """Split a slot capacity into <=512-wide column blocks of near-equal
    size (each >=256 unless cap is small), so every matmul stream is wide
    enough to hide its LDWEIGHTS under the previous matmul."""
    if cap <= 0:
        return []
    nb = -(-cap // 512)
    base = cap // nb
    rem = cap - base * nb
    return [base + (1 if i < rem else 0) for i in range(nb)]


def _blocks(capA, capB):
    """[(col0, ncols, slot)] covering [0, capA+capB) with slot 0/1."""
    blocks = []
    col = 0
    for slot, cap in ((0, capA), (1, capB)):
        for n in _split_cap(cap):
            blocks.append((col, n, slot))
            col += n
    return blocks


def _build(capA, capB):
    """Build + compile the uniform per-core kernel for slot caps."""
    import concourse.mybir as mybir
    import concourse.tile as tile
    from concourse import bacc

    bf16 = mybir.dt.bfloat16
    f32 = mybir.dt.float32

    M = capA + capB
    blocks = _blocks(capA, capB)
    NB = len(blocks)
    W1C = 2 * KD * P   # cols per stage-1 c-chunk (both slots)
    W2C = 2 * CH * P   # cols per stage-2 d-chunk (both slots)

    nc = bacc.Bacc("TRN2", target_bir_lowering=False, debug=False, num_devices=E)

    xt_d = nc.dram_tensor("xt", [P, KD * M], bf16, kind="ExternalInput")
    w1_d = nc.dram_tensor("w1", [P, CH * W1C], bf16, kind="ExternalInput")
    w2_d = nc.dram_tensor("w2", [P, DT * W2C], bf16, kind="ExternalInput")
    b1_d = nc.dram_tensor("b1s", [P, CH * 2], f32, kind="ExternalInput")
    y_d = nc.dram_tensor("y", [D, M], f32, kind="ExternalOutput")

    with tile.TileContext(nc) as tc:
        with (
            tc.tile_pool(name="xin", bufs=1) as xpool,
            tc.tile_pool(name="w1r", bufs=6) as w1pool,
            tc.tile_pool(name="w2r", bufs=3) as w2pool,
            tc.tile_pool(name="hbuf", bufs=1) as hpool,
            tc.tile_pool(name="obuf", bufs=4) as opool,
            tc.tile_pool(name="ps", bufs=min(8, 2 * NB), space="PSUM") as pspool,
        ):
            # Warm up the PE clock (HAM un-throttles after ~3.4us of
            # activity) with dummy matmuls while the first input DMAs
            # stream in; dummy gelu preloads the ACT table set.
            wz = xpool.tile([P, 512], bf16, tag="warmsrc")
            nc.vector.memset(wz[:], 0.0)
            pw = pspool.tile([P, 512], f32, tag="ps")
            for _ in range(N_WARM):
                nc.tensor.matmul(pw[:], wz[:, :P], wz[:], start=True, stop=True)
            wg = xpool.tile([P, 1], bf16, tag="warmgelu")
            nc.scalar.activation(wg[:], wz[:, :1],
                                 mybir.ActivationFunctionType.Gelu)

            # Input DMAs. sync queue order = priority: first c-chunk of
            # W1, then xt k-chunks, then the rest of W1 (ring-gated).
            xt = xpool.tile([P, KD * M], bf16, tag="xt")
            b1 = xpool.tile([P, CH * 2], f32, tag="b1")
            nc.scalar.dma_start(b1[:], b1_d.ap())
            for k in range(0, KD, 2):
                nc.sync.dma_start(xt[:, k * M:(k + 1) * M],
                                  xt_d.ap()[:, k * M:(k + 1) * M])
            ht = hpool.tile([P, CH, M], bf16, tag="ht")

            acts = []
            for c in range(CH):
                w1t = w1pool.tile([P, W1C], bf16, tag="w1c")
                # first two W1 chunks ride the scalar queue, in parallel
                # with xt on the sync queue, so c0/c1 can start early
                eng = nc.scalar if c < 2 else nc.sync
                eng.dma_start(w1t[:], w1_d.ap()[:, c * W1C:(c + 1) * W1C])
                if c == 0:
                    # odd xt k-chunks stream on the scalar queue in
                    # parallel with the even ones on sync
                    for k in range(1, KD, 2):
                        nc.scalar.dma_start(xt[:, k * M:(k + 1) * M],
                                            xt_d.ap()[:, k * M:(k + 1) * M])
                for bi, (col0, n, slot) in enumerate(blocks):
                    ps1 = pspool.tile([P, 512], f32, tag="ps", name="ps1")
                    for k in range(KD):
                        nc.tensor.matmul(
                            ps1[:, :n],
                            w1t[:, slot * KD * P + k * P:
                                slot * KD * P + (k + 1) * P],
                            xt[:, k * M + col0: k * M + col0 + n],
                            start=(k == 0),
                            stop=(k == KD - 1),
                        )
                        if c == 0 and bi == 0 and k < KD - 1:
                            # keep the PE busy (and the HAM clock warm)
                            # while the next xt k-chunk streams in
                            for _ in range(2):
                                nc.tensor.matmul(pw[:], wz[:, :P], wz[:],
                                                 start=True, stop=True)
                    act = nc.scalar.activation(
                        ht[:, c, col0:col0 + n], ps1[:, :n],
                        mybir.ActivationFunctionType.Gelu,
                        bias=b1[:, c * 2 + slot: c * 2 + slot + 1],
                    )
                    if bi == 0:
                        acts.append(act)

            # stage 2: y[d, m] = sum_h W2[h, d] * ht[h, m]
            for d in range(DT):
                w2t = w2pool.tile([P, W2C], bf16, tag="w2c")
                dma = nc.gpsimd.dma_start(w2t[:],
                                          w2_d.ap()[:, d * W2C:(d + 1) * W2C])
                # spread W2's 16.8MB across stage 1 so it never starves the
                # W1 stream: chunk d releases after gelu of c = 4 + 4d
                gate = acts[min(4 + 4 * d, CH - 2)]
                tile.add_dep_helper(dma.ins, gate.ins,
                                    reason="pace W2 DMA behind stage-1")
                for bi, (col0, n, slot) in enumerate(blocks):
                    ps2 = pspool.tile([P, 512], f32, tag="ps", name="ps2")
                    for h in range(CH):
                        nc.tensor.matmul(
                            ps2[:, :n],
                            w2t[:, slot * CH * P + h * P:
                                slot * CH * P + (h + 1) * P],
                            ht[:, h, col0:col0 + n],
                            start=(h == 0),
                            stop=(h == CH - 1),
                        )
                    ot = opool.tile([P, 512], f32, tag="ot")
                    nc.vector.tensor_copy(ot[:, :n], ps2[:, :n])
                    nc.sync.dma_start(y_d.ap()[d * P:(d + 1) * P,
                                               col0:col0 + n], ot[:, :n])
    nc.compile()
    return nc


def _route(x2d, Wg, bg):
    """fp64 gating: returns (top2 indices [N,2], renormalized weights [N,2])."""
    logits = x2d.astype(np.float64) @ Wg.astype(np.float64) + bg.astype(np.float64)
    m = logits.max(-1, keepdims=True)
    e = np.exp(logits - m)
    gates = e / e.sum(-1, keepdims=True)
    top2 = np.argsort(-gates, axis=-1, kind="stable")[:, :K]
    g2 = np.take_along_axis(gates, top2, axis=-1)
    w2 = g2 / np.maximum(g2.sum(-1, keepdims=True), 1e-12)
    return top2, w2


def _pack_w1(W1e, bf):
    # [D, H] -> [P, CH*KD*P], col = c*(KD*P) + k*P + h'
    return np.ascontiguousarray(
        W1e.astype(bf).reshape(KD, P, CH, P).transpose(1, 2, 0, 3)
        .reshape(P, CH * KD * P))


def _pack_w2(W2e, bf):
    # [H, D] -> [P, DT*CH*P], col = d*(CH*P) + h*P + d'
    return np.ascontiguousarray(
        W2e.astype(bf).reshape(CH, P, DT, P).transpose(1, 2, 0, 3)
        .reshape(P, DT * CH * P))


def kernel(x, Wg, bg, W1, b1, W2, b2, _run_opts=None):
    from concourse.bass_utils import run_bass_kernel_spmd

    x = np.asarray(x)
    x2d = x.reshape(N_TOK, D)
    top2, wgt2 = _route(x2d, np.asarray(Wg), np.asarray(bg))

    pos = [np.where((top2 == e).any(-1))[0] for e in range(E)]
    pw = [
        (wgt2 * (top2 == e))[pos[e]].sum(-1).astype(np.float32)
        for e in range(E)
    ]
    counts = np.array([len(p) for p in pos])
    order = np.argsort(-counts, kind="stable")
    bigs, smalls = order[:4], order[4:]
    capL = max(64, -(-int(counts[bigs[0]]) // 2))
    capS = max(64, -(-int(counts[smalls[0]]) // 2))
    M = capS + capL

    # slot 0 = small-expert slot (first columns), so the final stage-2
    # eviction + store is the smallest block
    if (capS, capL) not in _compiled_cache:
        _compiled_cache[(capS, capL)] = _build(capS, capL)
    nc = _compiled_cache[(capS, capL)]

    bf = ml_dtypes.bfloat16
    W1 = np.asarray(W1)
    W2 = np.asarray(W2)
    b1 = np.asarray(b1)
    b2 = np.asarray(b2)

    in_maps = []
    core_slots = []  # per core: [(expert, rows, weights, col0), ...]
    for i in range(4):
        eA, eB = int(bigs[i]), int(smalls[i])
        # shared per-pair weight images
        w1img = np.empty((P, CH * 2 * KD * P), bf)
        w1v = w1img.reshape(P, CH, 2, KD * P)
        w1v[:, :, 0, :] = _pack_w1(W1[eB], bf).reshape(P, CH, KD * P)
        w1v[:, :, 1, :] = _pack_w1(W1[eA], bf).reshape(P, CH, KD * P)
        w2img = np.empty((P, DT * 2 * CH * P), bf)
        w2v = w2img.reshape(P, DT, 2, CH * P)
        w2v[:, :, 0, :] = _pack_w2(W2[eB], bf).reshape(P, DT, CH * P)
        w2v[:, :, 1, :] = _pack_w2(W2[eA], bf).reshape(P, DT, CH * P)
        b1img = np.empty((P, CH * 2), np.float32)
        b1v = b1img.reshape(P, CH, 2)
        b1v[:, :, 0] = b1[eB].reshape(CH, P).T
        b1v[:, :, 1] = b1[eA].reshape(CH, P).T
        w1img = np.ascontiguousarray(w1img)
        w2img = np.ascontiguousarray(w2img)
        b1img = np.ascontiguousarray(b1img)

        hA = -(-len(pos[eA]) // 2)
        hB = -(-len(pos[eB]) // 2)
        for half in range(2):
            rowsA = pos[eA][:hA] if half == 0 else pos[eA][hA:]
            wA = pw[eA][:hA] if half == 0 else pw[eA][hA:]
            rowsB = pos[eB][:hB] if half == 0 else pos[eB][hB:]
            wB = pw[eB][:hB] if half == 0 else pw[eB][hB:]
            xtc = np.zeros((D, M), bf)
            xtc[:, :len(rowsB)] = x2d[rowsB].T
            xtc[:, capS:capS + len(rowsA)] = x2d[rowsA].T
            xtp = np.ascontiguousarray(
                xtc.reshape(KD, P, M).transpose(1, 0, 2).reshape(P, KD * M))
            in_maps.append({
                "xt": xtp,
                "w1": w1img,
                "w2": w2img,
                "b1s": b1img,
            })
            core_slots.append([(eB, rowsB, wB, 0), (eA, rowsA, wA, capS)])

    try:
        res = run_bass_kernel_spmd(nc, in_maps, core_ids=list(range(E)),
                                   **(_run_opts or {}))
    except Exception:
        # transient device errors (e.g. NRT_EXEC_UNIT_UNRECOVERABLE) have
        # been observed on this fabric; one retry usually clears them
        res = run_bass_kernel_spmd(nc, in_maps, core_ids=list(range(E)),
                                   **(_run_opts or {}))

    out = np.zeros((N_TOK, D), np.float32)
    for core in range(E):
        y = res.results[core]["y"]  # [D, M] fp32
        for e, rows, w, col0 in core_slots[core]:
            n = len(rows)
            if n == 0:
                continue
            out[rows] += (w[:, None] * y[:, col0:col0 + n].T
                          + w[:, None] * b2[e][None, :].astype(np.float32))
    if _run_opts is not None:
        kernel._last_result = res
    return out.reshape(B, T, D)


if __name__ == "__main__":
    rng = np.random.default_rng(0)
    ins = {
        "x": rng.standard_normal((B, T, D), dtype=np.float32),
        "Wg": rng.standard_normal((D, E), dtype=np.float32) * 0.03,
        "bg": rng.standard_normal((E,), dtype=np.float32) * 0.03,
        "W1": rng.standard_normal((E, D, H), dtype=np.float32) * 0.03,
        "b1": rng.standard_normal((E, H), dtype=np.float32) * 0.03,
        "W2": rng.standard_normal((E, H, D), dtype=np.float32) * 0.015,
        "b2": rng.standard_normal((E, D), dtype=np.float32) * 0.015,
    }
    out = kernel(**ins)
    print("kernel out:", out.shape, out.dtype, float(np.abs(out).mean()))
